# revision 1
# baseline (speedup 1.0000x reference)
"""Trainium2 Bass kernel for nn_FeatureContraction.

Computes out[b,c,w,x,v] = sum_i x[b,c,w,x,v,i] * node_attributes[b,c,i]
with B=C=128, X=3, Y=16 (wxv = 3*16*16 = 768, i = 16).

Strategy (8 NeuronCores, data-parallel over b):
  - each core owns 16 b-slices; x-shard is [16, 128, 768, 16] f32 (96 MiB)
  - SBUF layout: partitions = c (128), free = contiguous (wxv, i)
    -> DMA reads 48 KiB contiguous per partition (full HBM rate).
    The load casts f32 -> bf16 in the DMA datapath (SWDGE cast).
  - multiply: tmp[c, w, i] = x[c, w, i] * na[c, i] with a step-0
    broadcast AP on na (DVE 2x mode, contiguous streams).
  - reduce over i, split by w to balance engines:
      w < RED_SPLIT: DVE grouped tensor_reduce (innermost axis)
      w >= RED_SPLIT: 16 identity-weight PE matmuls accumulating the
      strided i-slices into PSUM, then ACT copies PSUM->SBUF.
  - the last b-slice is loaded in two halves so the pipeline tail is
    short (the DVE half finishes last).
This keeps the kernel at the HBM roofline (~100 MiB/core of traffic).
"""

import sys

for _p in ("/opt/trn_rl_repo",):
    if _p not in sys.path:
        sys.path.append(_p)

import numpy as np

import concourse.bass as bass
import concourse.mybir as mybir
import concourse.tile as tile
from concourse import bacc
from concourse.bass_utils import run_bass_kernel_spmd

# Problem dims (hardcoded per spec)
B, C, X, Y = 128, 128, 3, 16
WXV = X * Y * Y          # 768
I = Y                    # 16 (contraction axis)
N_CORES = 8
B_LOC = B // N_CORES     # 16 b-slices per core

RED_SPLIT = 336          # DVE reduces w < RED_SPLIT, PE reduces the rest

F32 = mybir.dt.float32
BF16 = mybir.dt.bfloat16

_COMPILED = None


def _build():
    nc = bacc.Bacc("TRN2", target_bir_lowering=False, debug=False,
                   num_devices=N_CORES)

    x_d = nc.dram_tensor("x", [B_LOC, C, WXV, I], F32, kind="ExternalInput")
    na_d = nc.dram_tensor("naT", [C, B_LOC, I], F32, kind="ExternalInput")
    eye_d = nc.dram_tensor("eye", [C, C], F32, kind="ExternalInput")
    out_d = nc.dram_tensor("out", [B_LOC, C, WXV], F32, kind="ExternalOutput")

    WA = RED_SPLIT
    WB = WXV - RED_SPLIT

    with tile.TileContext(nc) as tc:
        with (
            tc.tile_pool(name="const", bufs=1) as constp,
            tc.tile_pool(name="xp", bufs=4) as xp,
            tc.tile_pool(name="tmpp", bufs=3) as tmpp,
            tc.tile_pool(name="outp", bufs=3) as outp,
            tc.tile_pool(name="psp", bufs=4, space="PSUM") as psp,
        ):
            eye = constp.tile([C, C], BF16)
            na_sb = constp.tile([C, B_LOC, I], BF16)
            eye_f = constp.tile([C, C], F32)
            na_f = constp.tile([C, B_LOC, I], F32)

            def compute(b, xt_b, xt_a, oa_ap, ob_ap):
                nab = na_sb[:, b, :][:, None, :]
                # B half: mult then 16 PE identity matmuls (psum accumulate)
                tb = tmpp.tile([C, WB, I], BF16, tag="tmpb")
                nc.vector.tensor_mul(tb[:], xt_b,
                                     nab.broadcast_to([C, WB, I]))
                ps = psp.tile([C, WB], F32, tag="ps")
                for i in range(I):
                    nc.tensor.matmul(ps[:], eye[:], tb[:, :, i],
                                     start=(i == 0), stop=(i == I - 1))
                # A half: mult then DVE grouped reduce
                ta = tmpp.tile([C, WA, I], BF16, tag="tmpa")
                nc.vector.tensor_mul(ta[:], xt_a,
                                     nab.broadcast_to([C, WA, I]))
                nc.scalar.copy(ob_ap, ps[:])
                nc.vector.tensor_reduce(oa_ap, ta[:], mybir.AxisListType.X,
                                        mybir.AluOpType.add)

            for b in range(B_LOC - 1):
                xt = xp.tile([C, WXV, I], BF16, tag="x")
                nc.gpsimd.dma_start(xt[:], x_d[b])  # f32 -> bf16 cast
                if b == 0:
                    # constants via the idle HWDGE ring (keeps Q7 on x loads),
                    # converted to bf16 on DVE
                    nc.sync.dma_start(eye_f[:], eye_d[:])
                    nc.sync.dma_start(na_f[:], na_d[:])
                    nc.vector.tensor_copy(eye[:], eye_f[:])
                    nc.vector.tensor_copy(na_sb[:], na_f[:])
                ot = outp.tile([C, WXV], F32, tag="out")
                compute(b, xt[:, RED_SPLIT:, :], xt[:, :RED_SPLIT, :],
                        ot[:, :RED_SPLIT], ot[:, RED_SPLIT:])
                nc.scalar.dma_start(out_d[b], ot[:])

            # last b-slice: two half loads for a short pipeline tail
            b = B_LOC - 1
            xb = xp.tile([C, WB, I], BF16, tag="x")
            nc.gpsimd.dma_start(xb[:], x_d[b, :, RED_SPLIT:, :])
            xa = xp.tile([C, WA, I], BF16, tag="x")
            nc.gpsimd.dma_start(xa[:], x_d[b, :, :RED_SPLIT, :])
            ot = outp.tile([C, WXV], F32, tag="out")
            compute(b, xb[:], xa[:], ot[:, :RED_SPLIT], ot[:, RED_SPLIT:])
            nc.scalar.dma_start(out_d[b, :, RED_SPLIT:], ot[:, RED_SPLIT:])
            nc.scalar.dma_start(out_d[b, :, :RED_SPLIT], ot[:, :RED_SPLIT])

    nc.compile()
    return nc


def _get_compiled():
    global _COMPILED
    if _COMPILED is None:
        _COMPILED = _build()
    return _COMPILED


def _make_in_maps(inputs: dict):
    x = np.ascontiguousarray(np.asarray(inputs["x"], dtype=np.float32))
    na = np.asarray(inputs["node_attributes"], dtype=np.float32)

    x_sh = x.reshape(B, C, WXV, I)
    naT = np.ascontiguousarray(na.transpose(1, 0, 2))  # [C, B, I]
    eye = np.eye(C, dtype=np.float32)

    in_maps = []
    for k in range(N_CORES):
        b0 = k * B_LOC
        in_maps.append(
            {
                "x": x_sh[b0 : b0 + B_LOC],
                "naT": np.ascontiguousarray(naT[:, b0 : b0 + B_LOC, :]),
                "eye": eye,
            }
        )
    return in_maps


def _gather(results) -> np.ndarray:
    out = np.concatenate([r["out"] for r in results], axis=0)
    return out.reshape(B, C, X, Y, Y)


def _run(inputs: dict, trace: bool = False, trace_cores=None):
    in_maps = _make_in_maps(inputs)
    nc = _get_compiled()
    res = run_bass_kernel_spmd(
        nc,
        in_maps,
        core_ids=list(range(N_CORES)),
        trace=trace,
        trace_cores=trace_cores,
    )
    return _gather(res.results), res


def kernel(**inputs) -> np.ndarray:
    out, _ = _run(inputs, trace=False)
    return out



# revision 7
# speedup vs baseline: 2.5339x; 2.5339x over previous
"""Trainium2 Bass kernel for nn_FeatureContraction.

Computes out[b,c,w,x,v] = sum_i x[b,c,w,x,v,i] * node_attributes[b,c,i]
with B=C=128, X=3, Y=16 (wxv = 3*16*16 = 768, i = 16).

Strategy (8 NeuronCores, data-parallel over b; PE does ALL the math):
  - x is uploaded as fp8 e3m4 (4 mantissa bits; rel err ~1.4e-2 vs the
    2e-2 gate), host-packed so each b-slice is one [128, 12288] SBUF
    tile: partition p = (c32, i4) with c32 = c%32 within a 32-channel
    group, i4 = i%4 within an i-chunk; free axis = (g, k, w).
  - per (g, k): one matmul with a block-diagonal stationary
    S[(c32,i4), c32'] = delta * na[32g+c32, 4k+i4] (built on the host,
    [128, 32] bf16) and moving rhs x[(c32,i4), w]; the 4 i-chunks (k)
    accumulate in PSUM; output strip = psum partitions [32g, 32g+32).
    Group 3 (strip base 96) is inexpressible as an AP base partition
    (rust IR allows only 0/32/64), so it uses a [128, 64] stationary
    [0 | diag] at base 64 and is emitted FIRST: its start=True zeroes
    rows 64-95, which group 2's own start=True then overwrites.
  - so the whole contraction per b-slice = 32 matmuls into one
    [128, 768] f32 PSUM image; ACT copies it to bf16; HWDGE streams it
    out. DVE is not used at all.
  - HBM per core: 24 MiB x (fp8) + 2 MiB stationaries + 3 MiB out
    ~= 30 MB -> ~85 us at 358 GB/s/core; PE moving-data ~90 us.
"""

import os
import sys

for _p in ("/opt/trn_rl_repo",):
    if _p not in sys.path:
        sys.path.append(_p)

import ml_dtypes
import numpy as np

import concourse.bass as bass
import concourse.mybir as mybir
import concourse.tile as tile
from concourse import bacc
from concourse.bass_utils import run_bass_kernel_spmd

# Problem dims (hardcoded per spec)
B, C, X, Y = 128, 128, 3, 16
WXV = X * Y * Y          # 768
I = Y                    # 16 (contraction axis)
N_CORES = 8
B_LOC = B // N_CORES     # 16 b-slices per core

NG = 4                   # channel groups of 32 (PSUM col-strip aligned)
CG = C // NG             # 32 channels per group
NK = 4                   # i-chunks of 4: K = CG*4 = 128 partitions
IK = I // NK             # 4
W_HALF = WXV // 2        # 384 f32 per PSUM bank
NAD_COLS = 2 * CG + 3 * CG  # per (b, k): [g3 wide (64) | g0 | g1 | g2]

F32 = mybir.dt.float32
BF16 = mybir.dt.bfloat16
F8E3 = mybir.dt.float8e3

X_DT = os.environ.get("FC_X_DT", "f8e3")  # "f8e3" | "bf16" for A/B tests
X_MYBIR_DT = {"f8e3": F8E3, "bf16": BF16}[X_DT]
X_NP_DT = {"f8e3": ml_dtypes.float8_e3m4, "bf16": ml_dtypes.bfloat16}[X_DT]

_COMPILED = None


def _build():
    nc = bacc.Bacc("TRN2", target_bir_lowering=False, debug=False,
                   num_devices=N_CORES)

    x_d = nc.dram_tensor("x", [B_LOC, 128, NG * NK * WXV], X_MYBIR_DT,
                         kind="ExternalInput")
    nad_d = nc.dram_tensor("nad", [128, B_LOC * NK * NAD_COLS], BF16,
                           kind="ExternalInput")
    out_d = nc.dram_tensor("out", [B_LOC, C, WXV], BF16,
                           kind="ExternalOutput")

    with tile.TileContext(nc) as tc:
        with (
            tc.tile_pool(name="const", bufs=1) as constp,
            tc.tile_pool(name="xp", bufs=3) as xp,
            tc.tile_pool(name="outp", bufs=3) as outp,
            tc.tile_pool(name="psp", bufs=4, space="PSUM") as psp,
        ):
            nad_sb = constp.tile([128, B_LOC * NK * NAD_COLS], BF16)
            # stationaries via SWDGE ring so the first x load isn't queued
            # behind them on the HWDGE qSP ring
            nc.gpsimd.dma_start(nad_sb[:], nad_d[:])

            for b in range(B_LOC):
                xt = xp.tile([128, NG * NK * WXV], X_MYBIR_DT, tag="x")
                nc.sync.dma_start(xt[:], x_d[b])
                ot = outp.tile([C, WXV], BF16, tag="out")
                for h in range(2):
                    ps = psp.tile([128, W_HALF], F32, tag=f"ps{h}")
                    for g in (3, 0, 1, 2):  # wide group 3 first (see above)
                        for k in range(NK):
                            noff = (b * NK + k) * NAD_COLS
                            roff = (g * NK + k) * WXV + h * W_HALF
                            if g == 3:
                                lhsT = nad_sb[:, noff : noff + 2 * CG]
                                oap = ps[2 * CG : 4 * CG, :]
                            else:
                                s0 = noff + 2 * CG + g * CG
                                lhsT = nad_sb[:, s0 : s0 + CG]
                                oap = ps[CG * g : CG * (g + 1), :]
                            nc.tensor.matmul(
                                oap,
                                lhsT,
                                xt[:, roff : roff + W_HALF],
                                start=(k == 0),
                                stop=(k == NK - 1),
                            )
                    nc.scalar.copy(ot[:, h * W_HALF : (h + 1) * W_HALF],
                                   ps[:])
                nc.scalar.dma_start(out_d[b], ot[:])

    nc.compile()
    return nc


def _get_compiled():
    global _COMPILED
    if _COMPILED is None:
        _COMPILED = _build()
    return _COMPILED


def _make_in_maps(inputs: dict):
    x = np.asarray(inputs["x"], dtype=np.float32)
    na = np.asarray(inputs["node_attributes"], dtype=np.float32)

    # x[b, c, w, i] -> xq[b, p=(c32,i4), (g, k), w], cast first (cheaper
    # to transpose 1-2 B elems than 4 B)
    xq = x.reshape(B, C, WXV, I).astype(X_NP_DT)
    xq = xq.reshape(B, NG, CG, WXV, NK, IK)
    xq = np.ascontiguousarray(xq.transpose(0, 2, 5, 1, 4, 3))
    xq = xq.reshape(B, 128, NG * NK * WXV)

    # block-diag stationaries, per (b, k) a [128, NAD_COLS] block:
    #   cols [0, 64)    g3 wide: delta_{c32, j-32} * na[b, 96+c32, 4k+i4]
    #   cols [64+32g..) g in 0..2: delta_{c32, m} * na[b, 32g+c32, 4k+i4]
    tmp = na.reshape(B, NG, CG, NK, IK).transpose(2, 4, 0, 3, 1)
    tmp = tmp.astype(ml_dtypes.bfloat16)  # [c32, i4, b, k, g]
    nad = np.zeros((CG, IK, B, NK, NAD_COLS), dtype=ml_dtypes.bfloat16)
    ar = np.arange(CG)
    nad[ar, :, :, :, CG + ar] = tmp[ar, :, :, :, 3]
    for g in range(3):
        nad[ar, :, :, :, 2 * CG + g * CG + ar] = tmp[ar, :, :, :, g]
    nad = nad.reshape(128, B, NK * NAD_COLS)

    in_maps = []
    for kcore in range(N_CORES):
        b0 = kcore * B_LOC
        in_maps.append(
            {
                "x": xq[b0 : b0 + B_LOC],
                "nad": np.ascontiguousarray(
                    nad[:, b0 : b0 + B_LOC].reshape(128, -1)
                ),
            }
        )
    return in_maps


def _gather(results) -> np.ndarray:
    out = np.concatenate([np.asarray(r["out"]) for r in results], axis=0)
    return out.astype(np.float32).reshape(B, C, X, Y, Y)


def _run(inputs: dict, trace: bool = False, trace_cores=None):
    in_maps = _make_in_maps(inputs)
    nc = _get_compiled()
    res = run_bass_kernel_spmd(
        nc,
        in_maps,
        core_ids=list(range(N_CORES)),
        trace=trace,
        trace_cores=trace_cores,
    )
    return _gather(res.results), res


def kernel(**inputs) -> np.ndarray:
    out, _ = _run(inputs, trace=False)
    return out


# revision 10
# speedup vs baseline: 2.7851x; 1.0991x over previous
"""Trainium2 Bass kernel for nn_FeatureContraction.

Computes out[b,c,w,x,v] = sum_i x[b,c,w,x,v,i] * node_attributes[b,c,i]
with B=C=128, X=3, Y=16 (wxv = 3*16*16 = 768, i = 16).

Strategy (8 NeuronCores, data-parallel over b; PE does the math):
  - x is uploaded as fp8 e3m4 (4 mantissa bits; rel err ~1.4e-2 vs the
    2e-2 gate), host-packed so each b-slice is one [128, 12288] image:
    partition p = (c32, i4) with c32 = c%32 within a 32-channel group,
    i4 = i%4 within an i-chunk; free axis = (g, k, w). Loaded as four
    [128, 3072] per-group quarter tiles in consumption order.
  - per (g, k): one matmul with a block-diagonal stationary
    S[(c32,i4), c32'] = delta * na[32g+c32, 4k+i4] and moving rhs
    x[(c32,i4), w]; the 4 i-chunks (k) accumulate in PSUM; output
    strip = psum partitions [32g, 32g+32).
    Group 3 (strip base 96) is inexpressible as an AP base partition
    (rust IR allows only 0/32/64), so it uses a [128, 64] stationary
    [0 | diag] at base 64 and is emitted FIRST: its start=True zeroes
    rows 64-95, which group 2's own start=True then overwrites.
  - stationaries are built ON DEVICE by the (otherwise idle) DVE:
    S = mask * na_col, with mask in {mask32 [128,32], mask64 [128,64]}
    constants and na_col a per-partition scalar from a host-packed
    [128, 256] bf16 table (64 KB) -- so no 2.6 MB stationary upload.
  - whole contraction per b-slice = 32 matmuls into one [128, 768]
    f32 PSUM image; ACT copies it to bf16; HWDGE streams it out.
  - HBM per core: 24 MiB x (fp8) + 3 MiB out -> ~80 us at 358 GB/s;
    PE moving-data ~58 us busy. DMA-bound by design.
"""

import os
import sys

for _p in ("/opt/trn_rl_repo",):
    if _p not in sys.path:
        sys.path.append(_p)

import ml_dtypes
import numpy as np

import concourse.bass as bass
import concourse.mybir as mybir
import concourse.tile as tile
from concourse import bacc
from concourse.bass_utils import run_bass_kernel_spmd

# Problem dims (hardcoded per spec)
B, C, X, Y = 128, 128, 3, 16
WXV = X * Y * Y          # 768
I = Y                    # 16 (contraction axis)
N_CORES = 8
B_LOC = B // N_CORES     # 16 b-slices per core

NG = 4                   # channel groups of 32 (PSUM col-strip aligned)
CG = C // NG             # 32 channels per group
NK = 4                   # i-chunks of 4: K = CG*4 = 128 partitions
IK = I // NK             # 4
W_HALF = WXV // 2        # 384 f32 per PSUM bank
GQ = NK * WXV            # 3072: one group's x columns per b-slice
SKB = 2 * CG + 3 * CG    # 160 stationary cols per (b, k): [g3w|g0|g1|g2]

F32 = mybir.dt.float32
BF16 = mybir.dt.bfloat16
F8E3 = mybir.dt.float8e3

X_DT = os.environ.get("FC_X_DT", "f8e3")  # "f8e3" | "bf16" for A/B tests
X_MYBIR_DT = {"f8e3": F8E3, "bf16": BF16}[X_DT]
X_NP_DT = {"f8e3": ml_dtypes.float8_e3m4, "bf16": ml_dtypes.bfloat16}[X_DT]

GORDER = (3, 0, 1, 2)    # wide group 3 first (see module docstring)

_COMPILED = None


def _build():
    nc = bacc.Bacc("TRN2", target_bir_lowering=False, debug=False,
                   num_devices=N_CORES)

    x_d = nc.dram_tensor("x", [B_LOC, 128, NG * GQ], X_MYBIR_DT,
                         kind="ExternalInput")
    nacol_d = nc.dram_tensor("nacol", [128, B_LOC * NG * NK], F32,
                             kind="ExternalInput")
    mask_d = nc.dram_tensor("mask", [128, 3 * CG], BF16,
                            kind="ExternalInput")
    out_d = nc.dram_tensor("out", [B_LOC, C, WXV], BF16,
                           kind="ExternalOutput")

    with tile.TileContext(nc) as tc:
        with (
            tc.tile_pool(name="const", bufs=1) as constp,
            tc.tile_pool(name="xp", bufs=8) as xp,
            tc.tile_pool(name="sp", bufs=3) as sp,
            tc.tile_pool(name="outp", bufs=3) as outp,
            tc.tile_pool(name="psp", bufs=4, space="PSUM") as psp,
        ):
            nacol = constp.tile([128, B_LOC * NG * NK], F32)
            mask = constp.tile([128, 3 * CG], BF16)  # [mask64 | mask32]
            nc.sync.dma_start(mask[:], mask_d[:])
            nc.sync.dma_start(nacol[:], nacol_d[:])

            for b in range(B_LOC):
                # DVE builds this slice's stationaries from na_col table
                st = sp.tile([128, NK * SKB], BF16, tag="s")
                for k in range(NK):
                    for g in range(NG):
                        j = (b * NG + g) * NK + k
                        col = nacol[:, j : j + 1]
                        if g == 3:
                            nc.vector.tensor_scalar_mul(
                                st[:, k * SKB : k * SKB + 2 * CG],
                                mask[:, : 2 * CG], col)
                        else:
                            o = k * SKB + 2 * CG + g * CG
                            nc.vector.tensor_scalar_mul(
                                st[:, o : o + CG],
                                mask[:, 2 * CG : 3 * CG], col)

                xts = {}
                for g in GORDER:
                    xt = xp.tile([128, GQ], X_MYBIR_DT, tag="x")
                    nc.sync.dma_start(xt[:], x_d[b, :, g * GQ : (g + 1) * GQ])
                    xts[g] = xt

                ps0 = psp.tile([128, W_HALF], F32, tag="ps0")
                ps1 = psp.tile([128, W_HALF], F32, tag="ps1")
                ps = {0: ps0, 1: ps1}
                for g in GORDER:
                    for h in range(2):
                        for k in range(NK):
                            if g == 3:
                                lhsT = st[:, k * SKB : k * SKB + 2 * CG]
                                oap = ps[h][2 * CG : 4 * CG, :]
                            else:
                                s0 = k * SKB + 2 * CG + g * CG
                                lhsT = st[:, s0 : s0 + CG]
                                oap = ps[h][CG * g : CG * (g + 1), :]
                            nc.tensor.matmul(
                                oap,
                                lhsT,
                                xts[g][:, k * WXV + h * W_HALF :
                                       k * WXV + h * W_HALF + W_HALF],
                                start=(k == 0),
                                stop=(k == NK - 1),
                            )

                ot = outp.tile([C, WXV], BF16, tag="out")
                for h in range(2):
                    nc.scalar.copy(ot[:, h * W_HALF : (h + 1) * W_HALF],
                                   ps[h][:])
                nc.scalar.dma_start(out_d[b], ot[:])

    nc.compile()
    return nc


def _get_compiled():
    global _COMPILED
    if _COMPILED is None:
        _COMPILED = _build()
    return _COMPILED


def _make_in_maps(inputs: dict):
    x = np.asarray(inputs["x"], dtype=np.float32)
    na = np.asarray(inputs["node_attributes"], dtype=np.float32)

    # x[b, c, w, i] -> xq[b, p=(c32,i4), (g, k), w], cast first (cheaper
    # to transpose 1-2 B elems than 4 B)
    xq = x.reshape(B, C, WXV, I).astype(X_NP_DT)
    xq = xq.reshape(B, NG, CG, WXV, NK, IK)
    xq = np.ascontiguousarray(xq.transpose(0, 2, 5, 1, 4, 3))
    xq = xq.reshape(B, 128, NG * GQ)

    # na_col[p=(c32,i4), (b, g, k)] = na[b, 32g+c32, 4k+i4]
    nacol = na.reshape(B, NG, CG, NK, IK).transpose(2, 4, 0, 1, 3)
    nacol = np.ascontiguousarray(nacol).reshape(128, B * NG * NK)
    nacol = nacol.astype(np.float32)

    # masks: mask64[p, j] = (j >= 32) & (p//4 == j-32); mask32[p, m] = (p//4 == m)
    p4 = np.arange(128) // IK
    m32 = (p4[:, None] == np.arange(CG)[None, :])
    mask = np.concatenate(
        [np.zeros((128, CG), bool), m32, m32], axis=1
    ).astype(ml_dtypes.bfloat16)

    in_maps = []
    for kcore in range(N_CORES):
        b0 = kcore * B_LOC
        nci = nacol.reshape(128, B, NG * NK)[:, b0 : b0 + B_LOC]
        in_maps.append(
            {
                "x": xq[b0 : b0 + B_LOC],
                "nacol": np.ascontiguousarray(nci).reshape(128, -1),
                "mask": mask,
            }
        )
    return in_maps


def _gather(results) -> np.ndarray:
    out = np.concatenate([np.asarray(r["out"]) for r in results], axis=0)
    return out.astype(np.float32).reshape(B, C, X, Y, Y)


def _run(inputs: dict, trace: bool = False, trace_cores=None):
    in_maps = _make_in_maps(inputs)
    nc = _get_compiled()
    res = run_bass_kernel_spmd(
        nc,
        in_maps,
        core_ids=list(range(N_CORES)),
        trace=trace,
        trace_cores=trace_cores,
    )
    return _gather(res.results), res


def kernel(**inputs) -> np.ndarray:
    out, _ = _run(inputs, trace=False)
    return out


# revision 11
# speedup vs baseline: 2.8791x; 1.0338x over previous
"""Trainium2 Bass kernel for nn_FeatureContraction.

Computes out[b,c,w,x,v] = sum_i x[b,c,w,x,v,i] * node_attributes[b,c,i]
with B=C=128, X=3, Y=16 (wxv = 3*16*16 = 768, i = 16).

Strategy (8 NeuronCores, data-parallel over b; PE does the math):
  - x is uploaded as fp8 e3m4 (4 mantissa bits; rel err ~1.4e-2 vs the
    2e-2 gate), host-packed so each b-slice is one [128, 12288] image:
    partition p = (c32, i4) with c32 = c%32 within a 32-channel group,
    i4 = i%4 within an i-chunk; free axis = (g, k, w). Loaded as four
    [128, 3072] per-group quarter tiles in consumption order.
  - per (g, k): one matmul with a block-diagonal stationary
    S[(c32,i4), c32'] = delta * na[32g+c32, 4k+i4] and moving rhs
    x[(c32,i4), w]; the 4 i-chunks (k) accumulate in PSUM; output
    strip = psum partitions [32g, 32g+32).
    Group 3 (strip base 96) is inexpressible as an AP base partition
    (rust IR allows only 0/32/64), so it uses a [128, 64] stationary
    [0 | diag] at base 64 and is emitted FIRST: its start=True zeroes
    rows 64-95, which group 2's own start=True then overwrites.
  - stationaries are built ON DEVICE by the (otherwise idle) DVE:
    S = mask * na_col, with mask in {mask32 [128,32], mask64 [128,64]}
    constants and na_col a per-partition scalar from a host-packed
    [128, 256] bf16 table (64 KB) -- so no 2.6 MB stationary upload.
  - whole contraction per b-slice = 32 matmuls into one [128, 768]
    f32 PSUM image; ACT copies it to bf16; HWDGE streams it out.
  - HBM per core: 24 MiB x (fp8) + 3 MiB out -> ~80 us at 358 GB/s;
    PE moving-data ~58 us busy. DMA-bound by design.
"""

import os
import sys

for _p in ("/opt/trn_rl_repo",):
    if _p not in sys.path:
        sys.path.append(_p)

import ml_dtypes
import numpy as np

import concourse.bass as bass
import concourse.mybir as mybir
import concourse.tile as tile
from concourse import bacc
from concourse.bass_utils import run_bass_kernel_spmd

# Problem dims (hardcoded per spec)
B, C, X, Y = 128, 128, 3, 16
WXV = X * Y * Y          # 768
I = Y                    # 16 (contraction axis)
N_CORES = 8
B_LOC = B // N_CORES     # 16 b-slices per core

NG = 4                   # channel groups of 32 (PSUM col-strip aligned)
CG = C // NG             # 32 channels per group
NK = 4                   # i-chunks of 4: K = CG*4 = 128 partitions
IK = I // NK             # 4
W_HALF = WXV // 2        # 384 f32 per PSUM bank
GQ = NK * WXV            # 3072: one group's x columns per b-slice
SKB = 2 * CG + 3 * CG    # 160 stationary cols per (b, k): [g3w|g0|g1|g2]

F32 = mybir.dt.float32
BF16 = mybir.dt.bfloat16
F8E3 = mybir.dt.float8e3

X_DT = os.environ.get("FC_X_DT", "f8e3")  # "f8e3" | "bf16" for A/B tests
X_MYBIR_DT = {"f8e3": F8E3, "bf16": BF16}[X_DT]
X_NP_DT = {"f8e3": ml_dtypes.float8_e3m4, "bf16": ml_dtypes.bfloat16}[X_DT]

GORDER = (3, 0, 1, 2)    # wide group 3 first (see module docstring)

_COMPILED = None


def _build():
    nc = bacc.Bacc("TRN2", target_bir_lowering=False, debug=False,
                   num_devices=N_CORES)

    x_d = nc.dram_tensor("x", [B_LOC, 128, NG * GQ], X_MYBIR_DT,
                         kind="ExternalInput")
    nacol_d = nc.dram_tensor("nacol", [128, B_LOC * NG * NK], F32,
                             kind="ExternalInput")
    mask_d = nc.dram_tensor("mask", [128, 3 * CG], BF16,
                            kind="ExternalInput")
    out_d = nc.dram_tensor("out", [B_LOC, C, WXV], BF16,
                           kind="ExternalOutput")

    with tile.TileContext(nc) as tc:
        with (
            tc.tile_pool(name="const", bufs=1) as constp,
            tc.tile_pool(name="xp", bufs=8) as xp,
            tc.tile_pool(name="sp", bufs=3) as sp,
            tc.tile_pool(name="outp", bufs=3) as outp,
            tc.tile_pool(name="psp", bufs=4, space="PSUM") as psp,
        ):
            nacol = constp.tile([128, B_LOC * NG * NK], F32)
            mask = constp.tile([128, 3 * CG], BF16)  # [mask64 | mask32]
            # consts on the ACT HWDGE ring: keeps the sync ring free so
            # the first x quarter is the very first qSP transfer
            nc.scalar.dma_start(mask[:], mask_d[:])
            nc.scalar.dma_start(nacol[:], nacol_d[:])

            for b in range(B_LOC):
                # DVE builds this slice's stationaries from na_col table
                st = sp.tile([128, NK * SKB], BF16, tag="s")
                for k in range(NK):
                    for g in range(NG):
                        j = (b * NG + g) * NK + k
                        col = nacol[:, j : j + 1]
                        if g == 3:
                            nc.vector.tensor_scalar_mul(
                                st[:, k * SKB : k * SKB + 2 * CG],
                                mask[:, : 2 * CG], col)
                        else:
                            o = k * SKB + 2 * CG + g * CG
                            nc.vector.tensor_scalar_mul(
                                st[:, o : o + CG],
                                mask[:, 2 * CG : 3 * CG], col)

                xts = {}
                for g in GORDER:
                    xt = xp.tile([128, GQ], X_MYBIR_DT, tag="x")
                    nc.sync.dma_start(xt[:], x_d[b, :, g * GQ : (g + 1) * GQ])
                    xts[g] = xt

                ps0 = psp.tile([128, W_HALF], F32, tag="ps0")
                ps1 = psp.tile([128, W_HALF], F32, tag="ps1")
                ps = {0: ps0, 1: ps1}
                for g in GORDER:
                    for h in range(2):
                        for k in range(NK):
                            if g == 3:
                                lhsT = st[:, k * SKB : k * SKB + 2 * CG]
                                oap = ps[h][2 * CG : 4 * CG, :]
                            else:
                                s0 = k * SKB + 2 * CG + g * CG
                                lhsT = st[:, s0 : s0 + CG]
                                oap = ps[h][CG * g : CG * (g + 1), :]
                            nc.tensor.matmul(
                                oap,
                                lhsT,
                                xts[g][:, k * WXV + h * W_HALF :
                                       k * WXV + h * W_HALF + W_HALF],
                                start=(k == 0),
                                stop=(k == NK - 1),
                            )

                ot = outp.tile([C, WXV], BF16, tag="out")
                for h in range(2):
                    nc.scalar.copy(ot[:, h * W_HALF : (h + 1) * W_HALF],
                                   ps[h][:])
                nc.scalar.dma_start(out_d[b], ot[:])

    nc.compile()
    return nc


def _get_compiled():
    global _COMPILED
    if _COMPILED is None:
        _COMPILED = _build()
    return _COMPILED


def _make_in_maps(inputs: dict):
    x = np.asarray(inputs["x"], dtype=np.float32)
    na = np.asarray(inputs["node_attributes"], dtype=np.float32)

    # x[b, c, w, i] -> xq[b, p=(c32,i4), (g, k), w], cast first (cheaper
    # to transpose 1-2 B elems than 4 B)
    xq = x.reshape(B, C, WXV, I).astype(X_NP_DT)
    xq = xq.reshape(B, NG, CG, WXV, NK, IK)
    xq = np.ascontiguousarray(xq.transpose(0, 2, 5, 1, 4, 3))
    xq = xq.reshape(B, 128, NG * GQ)

    # na_col[p=(c32,i4), (b, g, k)] = na[b, 32g+c32, 4k+i4]
    nacol = na.reshape(B, NG, CG, NK, IK).transpose(2, 4, 0, 1, 3)
    nacol = np.ascontiguousarray(nacol).reshape(128, B * NG * NK)
    nacol = nacol.astype(np.float32)

    # masks: mask64[p, j] = (j >= 32) & (p//4 == j-32); mask32[p, m] = (p//4 == m)
    p4 = np.arange(128) // IK
    m32 = (p4[:, None] == np.arange(CG)[None, :])
    mask = np.concatenate(
        [np.zeros((128, CG), bool), m32, m32], axis=1
    ).astype(ml_dtypes.bfloat16)

    in_maps = []
    for kcore in range(N_CORES):
        b0 = kcore * B_LOC
        nci = nacol.reshape(128, B, NG * NK)[:, b0 : b0 + B_LOC]
        in_maps.append(
            {
                "x": xq[b0 : b0 + B_LOC],
                "nacol": np.ascontiguousarray(nci).reshape(128, -1),
                "mask": mask,
            }
        )
    return in_maps


def _gather(results) -> np.ndarray:
    out = np.concatenate([np.asarray(r["out"]) for r in results], axis=0)
    return out.astype(np.float32).reshape(B, C, X, Y, Y)


def _run(inputs: dict, trace: bool = False, trace_cores=None):
    in_maps = _make_in_maps(inputs)
    nc = _get_compiled()
    res = run_bass_kernel_spmd(
        nc,
        in_maps,
        core_ids=list(range(N_CORES)),
        trace=trace,
        trace_cores=trace_cores,
    )
    return _gather(res.results), res


def kernel(**inputs) -> np.ndarray:
    out, _ = _run(inputs, trace=False)
    return out


# revision 12
# speedup vs baseline: 2.9151x; 1.0125x over previous
"""Trainium2 Bass kernel for nn_FeatureContraction.

Computes out[b,c,w,x,v] = sum_i x[b,c,w,x,v,i] * node_attributes[b,c,i]
with B=C=128, X=3, Y=16 (wxv = 3*16*16 = 768, i = 16).

Strategy (8 NeuronCores, data-parallel over b; PE does the math):
  - x is uploaded as fp8 e3m4 (4 mantissa bits; rel err ~1.4e-2 vs the
    2e-2 gate), host-packed so each b-slice is one [128, 12288] image:
    partition p = (c32, i4) with c32 = c%32 within a 32-channel group,
    i4 = i%4 within an i-chunk; free axis = (g, k, w). Loaded as four
    [128, 3072] per-group quarter tiles in consumption order.
  - per (g, k): one matmul with a block-diagonal stationary
    S[(c32,i4), c32'] = delta * na[32g+c32, 4k+i4] and moving rhs
    x[(c32,i4), w]; the 4 i-chunks (k) accumulate in PSUM; output
    strip = psum partitions [32g, 32g+32).
    Group 3 (strip base 96) is inexpressible as an AP base partition
    (rust IR allows only 0/32/64), so it uses a [128, 64] stationary
    [0 | diag] at base 64 and is emitted FIRST: its start=True zeroes
    rows 64-95, which group 2's own start=True then overwrites.
  - stationaries are built ON DEVICE by the (otherwise idle) DVE:
    S = mask * na_col, with mask in {mask32 [128,32], mask64 [128,64]}
    constants and na_col a per-partition scalar from a host-packed
    [128, 256] bf16 table (64 KB) -- so no 2.6 MB stationary upload.
  - whole contraction per b-slice = 32 matmuls into one [128, 768]
    f32 PSUM image; ACT copies it to bf16; HWDGE streams it out.
  - HBM per core: 24 MiB x (fp8) + 3 MiB out -> ~80 us at 358 GB/s;
    PE moving-data ~58 us busy. DMA-bound by design.
"""

import os
import sys

for _p in ("/opt/trn_rl_repo",):
    if _p not in sys.path:
        sys.path.append(_p)

import ml_dtypes
import numpy as np

import concourse.bass as bass
import concourse.mybir as mybir
import concourse.tile as tile
from concourse import bacc
from concourse.bass_utils import run_bass_kernel_spmd

# Problem dims (hardcoded per spec)
B, C, X, Y = 128, 128, 3, 16
WXV = X * Y * Y          # 768
I = Y                    # 16 (contraction axis)
N_CORES = 8
B_LOC = B // N_CORES     # 16 b-slices per core

NG = 4                   # channel groups of 32 (PSUM col-strip aligned)
CG = C // NG             # 32 channels per group
NK = 4                   # i-chunks of 4: K = CG*4 = 128 partitions
IK = I // NK             # 4
W_HALF = WXV // 2        # 384 f32 per PSUM bank
GQ = NK * WXV            # 3072: one group's x columns per b-slice
SKB = 2 * CG + 3 * CG    # 160 stationary cols per (b, k): [g3w|g0|g1|g2]

F32 = mybir.dt.float32
BF16 = mybir.dt.bfloat16
F8E3 = mybir.dt.float8e3

X_DT = os.environ.get("FC_X_DT", "f8e3")  # "f8e3" | "bf16" for A/B tests
X_MYBIR_DT = {"f8e3": F8E3, "bf16": BF16}[X_DT]
X_NP_DT = {"f8e3": ml_dtypes.float8_e3m4, "bf16": ml_dtypes.bfloat16}[X_DT]

GORDER = (3, 0, 1, 2)    # wide group 3 first (see module docstring)

_COMPILED = None


def _build():
    nc = bacc.Bacc("TRN2", target_bir_lowering=False, debug=False,
                   num_devices=N_CORES)

    x_d = nc.dram_tensor("x", [B_LOC, 128, NG * GQ], X_MYBIR_DT,
                         kind="ExternalInput")
    nacol_d = nc.dram_tensor("nacol", [128, B_LOC * NG * NK], F32,
                             kind="ExternalInput")
    mask_d = nc.dram_tensor("mask", [128, 3 * CG], BF16,
                            kind="ExternalInput")
    out_d = nc.dram_tensor("out", [B_LOC, C, WXV], BF16,
                           kind="ExternalOutput")

    with tile.TileContext(nc) as tc:
        with (
            tc.tile_pool(name="const", bufs=1) as constp,
            tc.tile_pool(name="xp", bufs=8) as xp,
            tc.tile_pool(name="sp", bufs=3) as sp,
            tc.tile_pool(name="outp", bufs=3) as outp,
            tc.tile_pool(name="psp", bufs=4, space="PSUM") as psp,
        ):
            nacol = constp.tile([128, B_LOC * NG * NK], F32)
            mask = constp.tile([128, 3 * CG], BF16)  # [mask64 | mask32]
            # consts on the ACT HWDGE ring: keeps the sync ring free so
            # the first x quarter is the very first qSP transfer
            nc.scalar.dma_start(mask[:], mask_d[:])
            nc.scalar.dma_start(nacol[:], nacol_d[:])

            for b in range(B_LOC):
                # DVE builds this slice's stationaries from na_col table
                st = sp.tile([128, NK * SKB], BF16, tag="s")
                for g in GORDER:  # g3's sections first: PE needs them first
                    for k in range(NK):
                        j = (b * NG + g) * NK + k
                        col = nacol[:, j : j + 1]
                        if g == 3:
                            nc.vector.tensor_scalar_mul(
                                st[:, k * SKB : k * SKB + 2 * CG],
                                mask[:, : 2 * CG], col)
                        else:
                            o = k * SKB + 2 * CG + g * CG
                            nc.vector.tensor_scalar_mul(
                                st[:, o : o + CG],
                                mask[:, 2 * CG : 3 * CG], col)

                xts = {}
                for g in GORDER:
                    xt = xp.tile([128, GQ], X_MYBIR_DT, tag="x")
                    nc.sync.dma_start(xt[:], x_d[b, :, g * GQ : (g + 1) * GQ])
                    xts[g] = xt

                ps0 = psp.tile([128, W_HALF], F32, tag="ps0")
                ps1 = psp.tile([128, W_HALF], F32, tag="ps1")
                ps = {0: ps0, 1: ps1}
                for g in GORDER:
                    for h in range(2):
                        for k in range(NK):
                            if g == 3:
                                lhsT = st[:, k * SKB : k * SKB + 2 * CG]
                                oap = ps[h][2 * CG : 4 * CG, :]
                            else:
                                s0 = k * SKB + 2 * CG + g * CG
                                lhsT = st[:, s0 : s0 + CG]
                                oap = ps[h][CG * g : CG * (g + 1), :]
                            nc.tensor.matmul(
                                oap,
                                lhsT,
                                xts[g][:, k * WXV + h * W_HALF :
                                       k * WXV + h * W_HALF + W_HALF],
                                start=(k == 0),
                                stop=(k == NK - 1),
                            )

                ot = outp.tile([C, WXV], BF16, tag="out")
                for h in range(2):
                    nc.scalar.copy(ot[:, h * W_HALF : (h + 1) * W_HALF],
                                   ps[h][:])
                nc.scalar.dma_start(out_d[b], ot[:])

    nc.compile()
    return nc


def _get_compiled():
    global _COMPILED
    if _COMPILED is None:
        _COMPILED = _build()
    return _COMPILED


def _make_in_maps(inputs: dict):
    x = np.asarray(inputs["x"], dtype=np.float32)
    na = np.asarray(inputs["node_attributes"], dtype=np.float32)

    # x[b, c, w, i] -> xq[b, p=(c32,i4), (g, k), w], cast first (cheaper
    # to transpose 1-2 B elems than 4 B)
    xq = x.reshape(B, C, WXV, I).astype(X_NP_DT)
    xq = xq.reshape(B, NG, CG, WXV, NK, IK)
    xq = np.ascontiguousarray(xq.transpose(0, 2, 5, 1, 4, 3))
    xq = xq.reshape(B, 128, NG * GQ)

    # na_col[p=(c32,i4), (b, g, k)] = na[b, 32g+c32, 4k+i4]
    nacol = na.reshape(B, NG, CG, NK, IK).transpose(2, 4, 0, 1, 3)
    nacol = np.ascontiguousarray(nacol).reshape(128, B * NG * NK)
    nacol = nacol.astype(np.float32)

    # masks: mask64[p, j] = (j >= 32) & (p//4 == j-32); mask32[p, m] = (p//4 == m)
    p4 = np.arange(128) // IK
    m32 = (p4[:, None] == np.arange(CG)[None, :])
    mask = np.concatenate(
        [np.zeros((128, CG), bool), m32, m32], axis=1
    ).astype(ml_dtypes.bfloat16)

    in_maps = []
    for kcore in range(N_CORES):
        b0 = kcore * B_LOC
        nci = nacol.reshape(128, B, NG * NK)[:, b0 : b0 + B_LOC]
        in_maps.append(
            {
                "x": xq[b0 : b0 + B_LOC],
                "nacol": np.ascontiguousarray(nci).reshape(128, -1),
                "mask": mask,
            }
        )
    return in_maps


def _gather(results) -> np.ndarray:
    out = np.concatenate([np.asarray(r["out"]) for r in results], axis=0)
    return out.astype(np.float32).reshape(B, C, X, Y, Y)


def _run(inputs: dict, trace: bool = False, trace_cores=None):
    in_maps = _make_in_maps(inputs)
    nc = _get_compiled()
    res = run_bass_kernel_spmd(
        nc,
        in_maps,
        core_ids=list(range(N_CORES)),
        trace=trace,
        trace_cores=trace_cores,
    )
    return _gather(res.results), res


def kernel(**inputs) -> np.ndarray:
    out, _ = _run(inputs, trace=False)
    return out
